# revision 1
# baseline (speedup 1.0000x reference)
"""Trainium2 Bass kernel for nn_ConvAttention_34600256537137.

Math notes (validated against the reference to ~3e-6 rel err):
  qkv = 1x1conv(x, w1)+b1 -> Q,K,V;  score = conv5x5(Q_s)+conv5x5(K_t)+b2;
  attn = softmax_t(score);  out = einsum(attn, V).
  Softmax over t is shift-invariant, so the Q-half of the score (constant in
  t), b2, and the K-path bias all cancel.  The computation collapses to:
    weff[ci,dy,dx] = sum_c w1K[c,ci] * w2K[c,dy,dx]        (host, tiny)
    sK[b,t,h,w]    = conv5x5_reflect(x[b,:,:,:,t], weff)
    e = exp(sK);  den = sum_t e
    out[b,o,h,w,s] = (sum_{ci,t} w1V[o,ci] * e * x) / den + b1V[o]
  (s-independent; normalization folded to the end; bias added on host)

Sharding: 8 cores = (b in {0,1}) x (4 chunks of 8 rows of H).  All reflect
padding and layout transforms are precomputed host-side so every core runs an
identical program on its own slices.

Perf structure (v3):
  - DMAs are spread round-robin over all 5 engine queues (a single queue
    serializes descriptors at ~600ns each).
  - slab arrives as 6 row-pair tiles so conv matmuls start as rows land.
  - score conv: T[tap,pos] = weff^T @ slab streamed once (24 half-row
    matmuls); PSUM->SBUF copies write T as (tap, t, row, w); DRAM bounce +
    25 per-tap gathers build R[(t,h), tap, w]; DVE reduces taps on 128 lanes.
  - softmax denominator via indicator-matmul on PE (no partition reduce);
    normalization happens on the final PSUM->SBUF read.
"""

import sys

if "/opt/trn_rl_repo" not in sys.path:
    sys.path.insert(0, "/opt/trn_rl_repo")

import numpy as np

B, C, H, W, S = 2, 64, 32, 32, 16
KS, PAD = 5, 2
NCORES = 8
ROWS = H // 4            # output rows per core
SLAB_R = ROWS + 2 * PAD  # 12
SLAB_W = W + 2 * PAD     # 36
NTAP = KS * KS           # 25
NPOS = SLAB_R * SLAB_W * S  # 6912 slab positions
HW = ROWS * W            # 256 output positions

_MODULE = None


def _build_module():
    import concourse.bacc as bacc
    import concourse.bass as bass
    import concourse.tile as tile
    from concourse import mybir

    f32 = mybir.dt.float32
    AF = mybir.ActivationFunctionType
    ALU = mybir.AluOpType
    nc = bacc.Bacc("TRN2", target_bir_lowering=False, debug=False, num_devices=NCORES)

    slab_d = nc.dram_tensor("slab", [C, SLAB_R, SLAB_W, S], f32, kind="ExternalInput")
    xt_d = nc.dram_tensor("xt", [128, 8, HW], f32, kind="ExternalInput")
    weff_d = nc.dram_tensor("weff", [C, NTAP], f32, kind="ExternalInput")
    w1vr_d = nc.dram_tensor("w1vr", [128, 8, C], f32, kind="ExternalInput")
    hsel_d = nc.dram_tensor("hsel", [128, ROWS], f32, kind="ExternalInput")
    o_d = nc.dram_tensor("o", [C, S, HW], f32, kind="ExternalOutput")

    # scratch DRAM for partition-crossing rearrangements
    td_d = nc.dram_tensor("td", [NTAP, S, SLAB_R, SLAB_W], f32)   # T, t-major
    ed_d = nc.dram_tensor("ed", [S, ROWS, W], f32)                # exp(sK), t-major
    dend_d = nc.dram_tensor("dend", [ROWS * W], f32)              # 1/den, flat hw

    engs = None
    _rr = [0]

    def dma(out, in_):
        e = engs[_rr[0] % len(engs)]
        _rr[0] += 1
        e.dma_start(out, in_)

    with tile.TileContext(nc) as tc:
        engs = [nc.sync, nc.scalar, nc.gpsimd]
        with tc.tile_pool(name="sb", bufs=1) as sb, tc.tile_pool(
            name="ps", bufs=6, space="PSUM"
        ) as ps, tc.tile_pool(name="pso", bufs=1, space="PSUM") as pso:
            # --- loads: weff tiny on gpsimd; slab pairs lead sync/scalar ---
            s_weff = sb.tile([C, NTAP], f32)
            nc.gpsimd.dma_start(s_weff, weff_d.ap())
            slab_t = []
            for rp in range(6):
                st = sb.tile([C, 2, SLAB_W, S], f32, tag=f"slab{rp}")
                slab_t.append(st)
                (nc.sync, nc.scalar, nc.gpsimd)[rp % 3].dma_start(
                    st, slab_d.ap()[:, 2 * rp : 2 * rp + 2, :, :]
                )
            s_hsel = sb.tile([128, ROWS], f32)
            nc.gpsimd.dma_start(s_hsel, hsel_d.ap())
            s_xt = sb.tile([128, 8, HW], f32)
            nc.sync.dma_start(s_xt, xt_d.ap())
            s_w1vr = sb.tile([128, 8, C], f32)
            nc.scalar.dma_start(s_w1vr, w1vr_d.ap())

            # --- phase 1: T[tap, (row, w, t)] = weff^T @ slab, half-row chunks
            # s_T2 holds T transposed to (tap, t, row, w): w contiguous.
            s_T2 = sb.tile([NTAP, S, SLAB_R, SLAB_W], f32)
            HREST = SLAB_W // 2  # 18
            for hr in range(SLAB_R * 2):
                row, half = divmod(hr, 2)
                p_t = ps.tile([NTAP, HREST, S], f32, tag="pt")
                nc.tensor.matmul(
                    p_t,
                    s_weff,
                    slab_t[row // 2][:, row % 2, half * HREST : (half + 1) * HREST, :],
                    start=True,
                    stop=True,
                )
                # copy PSUM -> s_T2[(tap), t, row, w-half] (strided write)
                eng = nc.vector if hr % 2 == 0 else nc.scalar
                if eng is nc.vector:
                    eng.tensor_copy(
                        s_T2[:, :, row, half * HREST : (half + 1) * HREST],
                        p_t.transpose([0, 2, 1]),
                    )
                else:
                    eng.copy(
                        s_T2[:, :, row, half * HREST : (half + 1) * HREST],
                        p_t.transpose([0, 2, 1]),
                    )

            # --- T to DRAM (contiguous both sides), 6 row-pair chunks so
            # gathers can pipeline behind the conv copies ---
            for ci in range(6):
                e = (nc.sync, nc.scalar, nc.gpsimd)[ci % 3]
                e.dma_start(
                    td_d.ap()[:, :, 2 * ci : 2 * ci + 2, :],
                    s_T2[:, :, 2 * ci : 2 * ci + 2, :],
                )

            # --- 25 per-tap gathers into R[(t,h) 128p, tap, w] ---
            s_R = sb.tile([128, NTAP, W], f32)
            for k in range(NTAP):
                dyi, dxi = divmod(k, KS)
                src = bass.AP(
                    tensor=td_d.ap().tensor,
                    offset=k * NPOS + dyi * SLAB_W + dxi,
                    ap=[[SLAB_R * SLAB_W, S], [SLAB_W, ROWS], [1, W]],
                )
                dma(s_R[:, k, :], src)

            # --- tap reduce on 128 lanes (strided view puts tap innermost) ---
            s_sk = sb.tile([128, W], f32)  # [(t,h), w]
            nc.vector.tensor_reduce(
                s_sk, s_R.transpose([0, 2, 1]), axis=mybir.AxisListType.X, op=ALU.add
            )

            # --- e = exp(sK) in [(t,h), w]; den via indicator-matmul on PE ---
            s_e = sb.tile([128, W], f32)
            nc.scalar.activation(s_e, s_sk, AF.Exp)
            p_den = pso.tile([ROWS, W], f32, tag="den")
            nc.tensor.matmul(p_den, s_hsel, s_e, start=True, stop=True)
            s_rcp = sb.tile([ROWS, W], f32)
            nc.vector.reciprocal(s_rcp, p_den)
            nc.scalar.dma_start(dend_d.ap(), s_rcp)
            s_rcpb = sb.tile([C, HW], f32)
            nc.scalar.dma_start(
                s_rcpb,
                bass.AP(tensor=dend_d.ap().tensor, offset=0, ap=[[0, C], [1, HW]]),
            )

            # --- bounce e to [t, hw] and read back as [(ci8,t), hw] ---
            # (s_e partitions iterate (t, h) so the flat [t, h, w] layout of
            # ed_d matches the source order directly)
            nc.sync.dma_start(ed_d.ap(), s_e)
            s_eb = sb.tile([128, HW], f32)
            for g in range(8):
                src = bass.AP(
                    tensor=ed_d.ap().tensor,
                    offset=0,
                    ap=[[ROWS * W, S], [1, ROWS * W]],  # (t, hw)
                )
                (nc.sync if g % 2 == 0 else nc.scalar).dma_start(
                    s_eb[g * S : (g + 1) * S, :], src
                )

            # --- V path: xattn = x_t * e; contract (ci,t) on PE ---
            s_xa = sb.tile([128, 8, HW], f32)
            nc.vector.tensor_tensor(
                s_xa,
                s_xt,
                s_eb.unsqueeze(1).broadcast_to((128, 8, HW)),
                op=ALU.mult,
            )
            p_o = pso.tile([C, HW], f32, tag="out")
            for g in range(8):
                nc.tensor.matmul(
                    p_o,
                    s_w1vr[:, g, :],
                    s_xa[:, g, :],
                    start=(g == 0),
                    stop=(g == 7),
                )
            # normalize on the PSUM->SBUF read
            s_o = sb.tile([C, HW], f32)
            nc.vector.tensor_tensor(s_o, p_o, s_rcpb, op=ALU.mult)
            bounds = [0, 22, 43, C]
            for ci, e in enumerate((nc.sync, nc.scalar, nc.gpsimd)):
                a, b = bounds[ci], bounds[ci + 1]
                e.dma_start(
                    o_d.ap()[a:b],
                    s_o[a:b].unsqueeze(1).broadcast_to((b - a, S, HW)),
                )

    nc.compile()
    return nc


def _get_module():
    global _MODULE
    if _MODULE is None:
        _MODULE = _build_module()
    return _MODULE


def make_host_inputs(x, w1, b1, w2, b2):
    """Host-side precompute: folded weights + per-core reflect-padded slices."""
    x = np.ascontiguousarray(np.asarray(x, np.float32))
    w1 = np.asarray(w1, np.float32)
    w2 = np.asarray(w2, np.float32)

    w1K = w1[C : 2 * C, :, 0, 0]          # [c, ci]
    w2K = w2[0, C : 2 * C]                # [c, 5, 5]
    weff = np.ascontiguousarray(
        np.einsum("ci,cyx->iyx", w1K, w2K).reshape(C, NTAP)
    )
    w1V = w1[2 * C :, :, 0, 0]            # [co, ci]

    # w1vr[(ci8,t), g, co] = w1V[co, 8g+ci8]
    tmp = w1V.T.reshape(8, 8, C)                      # (g, ci8, co)
    w1vr = np.ascontiguousarray(
        np.broadcast_to(tmp[:, :, None, :], (8, 8, S, C))
        .transpose(1, 2, 0, 3)
        .reshape(128, 8, C)
    )

    # hsel[(t,h), m] = 1 if h == m  (partition index = t*ROWS + h)
    hsel = np.zeros((128, ROWS), np.float32)
    for t in range(S):
        for h in range(ROWS):
            hsel[t * ROWS + h, h] = 1.0

    in_maps = []
    for core in range(NCORES):
        b, hc = divmod(core, 4)
        h0 = ROWS * hc
        xp = np.pad(x[b], ((0, 0), (PAD, PAD), (PAD, PAD), (0, 0)), mode="reflect")
        slab = np.ascontiguousarray(xp[:, h0 : h0 + SLAB_R, :, :])
        xs = x[b][:, h0 : h0 + ROWS, :, :]            # [ci, h, w, t]
        xt = np.ascontiguousarray(
            xs.reshape(8, 8, ROWS, W, S)
            .transpose(1, 4, 0, 2, 3)
            .reshape(128, 8, HW)
        )
        in_maps.append(
            {"slab": slab, "xt": xt, "weff": weff, "w1vr": w1vr, "hsel": hsel}
        )
    return in_maps


def assemble_output(results, b1):
    b1V = np.asarray(b1, np.float32)[2 * C :]
    out = np.empty((B, C, H, W, S), np.float32)
    for core in range(NCORES):
        b, hc = divmod(core, 4)
        h0 = ROWS * hc
        o = results[core]["o"].reshape(C, S, ROWS, W).transpose(0, 2, 3, 1)
        out[b, :, h0 : h0 + ROWS, :, :] = o
    out += b1V[None, :, None, None, None]
    return out


def kernel(x, w1, b1, w2, b2):
    from concourse.bass_utils import run_bass_kernel_spmd

    nc = _get_module()
    in_maps = make_host_inputs(x, w1, b1, w2, b2)
    res = run_bass_kernel_spmd(nc, in_maps, core_ids=list(range(NCORES)))
    return assemble_output(res.results, b1)



# revision 3
# speedup vs baseline: 1.5357x; 1.5357x over previous
"""Trainium2 Bass kernel for nn_ConvAttention_34600256537137.

Math notes (validated against the reference):
  qkv = 1x1conv(x, w1)+b1 -> Q,K,V;  score = conv5x5(Q_s)+conv5x5(K_t)+b2;
  attn = softmax_t(score);  out = einsum(attn, V).
  Softmax over t is shift-invariant, so the Q-half of the score (constant in
  t), b2, and the K-path bias all cancel.  The computation collapses to:
    weff[ci,dy,dx] = sum_c w1K[c,ci] * w2K[c,dy,dx]        (host, tiny)
    sK[b,t,h,w]    = conv5x5_reflect(x[b,:,:,:,t], weff)
    e = exp(sK);  den = sum_t e
    out[b,o,h,w,s] = (sum_{ci,t} w1V[o,ci] * (e/den) * x) + b1V[o]
  (s-independent; the S-broadcast and +b1V happen on host)

Sharding: 8 cores = (b in {0,1}) x (4 chunks of 8 rows of H).

Perf structure (v4):
  - matmul operands are bf16: 1 cycle/row on PE (fp32 is 4) and half the
    HBM bytes for the two big loads (slab, xt).
  - conv: T[tap,(t,w)] = weff^T @ slab per half-row; PSUM->SBUF copies into
    s_T[tap, r, t, w]; contiguous row-pair stores to td; TWO 5-dim-AP
    gathers build s_R[(h,t), tap, w]; DVE reduces taps.
  - spatial partition packing is (h,t): h=p//16, t=p%16.  That makes
    1/den broadcast (over t) and e replication (over ci8) pure indicator
    matmuls on the PE - no DRAM bounce for either.
  - e is normalized by 1/den BEFORE the V path, so the output needs no
    final normalization.
  - all critical DMAs ride the two HW DGE queues (sync/scalar); gpsimd
    (software DGE) only carries small early constant loads.
"""

import sys

if "/opt/trn_rl_repo" not in sys.path:
    sys.path.insert(0, "/opt/trn_rl_repo")

import numpy as np

B, C, H, W, S = 2, 64, 32, 32, 16
KS, PAD = 5, 2
NCORES = 8
ROWS = H // 4            # output rows per core (8)
SLAB_R = ROWS + 2 * PAD  # 12
SLAB_W = W + 2 * PAD     # 36
NTAP = KS * KS           # 25
HW = ROWS * W            # 256 output positions
HREST = SLAB_W // 2      # 18

_MODULE = None


def _build_module():
    import concourse.bacc as bacc
    import concourse.bass as bass
    import concourse.tile as tile
    from concourse import mybir

    f32 = mybir.dt.float32
    bf16 = mybir.dt.bfloat16
    AF = mybir.ActivationFunctionType
    ALU = mybir.AluOpType
    nc = bacc.Bacc("TRN2", target_bir_lowering=False, debug=False, num_devices=NCORES)

    slab_d = nc.dram_tensor("slab", [C, SLAB_R, SLAB_W, S], bf16, kind="ExternalInput")
    xt_d = nc.dram_tensor("xt", [128, 8, HW], bf16, kind="ExternalInput")
    weff_d = nc.dram_tensor("weff", [C, NTAP], bf16, kind="ExternalInput")
    w1vr_d = nc.dram_tensor("w1vr", [128, 8, C], bf16, kind="ExternalInput")
    hsel_d = nc.dram_tensor("hsel", [128, ROWS], f32, kind="ExternalInput")
    irx_d = nc.dram_tensor("irx", [ROWS, 128], f32, kind="ExternalInput")
    irep_d = nc.dram_tensor("irep", [S, 128], bf16, kind="ExternalInput")
    o_d = nc.dram_tensor("o", [C, HW], f32, kind="ExternalOutput")

    # scratch DRAM for partition-crossing rearrangements
    td_d = nc.dram_tensor("td", [NTAP, SLAB_R, S, SLAB_W], bf16)  # T, tap-major
    ed_d = nc.dram_tensor("ed", [S, ROWS, W], bf16)               # e/den, t-major

    with tile.TileContext(nc) as tc:
        with tc.tile_pool(name="sb", bufs=1) as sb, tc.tile_pool(
            name="ps", bufs=4, space="PSUM"
        ) as ps, tc.tile_pool(name="pso", bufs=1, space="PSUM") as pso:
            # --- loads: weff first (needed by first matmul), slab row pairs
            # alternate sync/scalar; constants ride gpsimd (software DGE) ---
            s_weff = sb.tile([C, NTAP], bf16)
            nc.sync.dma_start(s_weff, weff_d.ap())
            slab_t = []
            for rp in range(6):
                st = sb.tile([C, 2, SLAB_W, S], bf16, tag=f"slab{rp}")
                slab_t.append(st)
                (nc.sync, nc.scalar)[rp % 2].dma_start(
                    st, slab_d.ap()[:, 2 * rp : 2 * rp + 2, :, :]
                )
            s_hsel = sb.tile([128, ROWS], f32)
            nc.gpsimd.dma_start(s_hsel, hsel_d.ap())
            s_irx = sb.tile([ROWS, 128], f32)
            nc.gpsimd.dma_start(s_irx, irx_d.ap())
            s_irep = sb.tile([S, 128], bf16)
            nc.gpsimd.dma_start(s_irep, irep_d.ap())
            s_w1vr = sb.tile([128, 8, C], bf16)
            nc.gpsimd.dma_start(s_w1vr, w1vr_d.ap())
            s_xt = sb.tile([128, 8, HW], bf16)
            nc.scalar.dma_start(s_xt, xt_d.ap())

            # --- phase 1: T = weff^T @ slab, half-row chunks, streamed in
            # (t, w) order so the PSUM->SBUF copy is a plain copy ---
            s_T = sb.tile([NTAP, SLAB_R, S, SLAB_W], bf16)
            for hr in range(SLAB_R * 2):
                row, half = divmod(hr, 2)
                p_t = ps.tile([NTAP, S, HREST], f32, tag="pt")
                rhs = slab_t[row // 2][
                    :, row % 2, half * HREST : (half + 1) * HREST, :
                ].transpose([0, 2, 1])
                nc.tensor.matmul(p_t, s_weff, rhs, start=True, stop=True)
                eng = nc.vector if hr % 2 == 0 else nc.scalar
                dst = s_T[:, row, :, half * HREST : (half + 1) * HREST]
                if eng is nc.vector:
                    eng.tensor_copy(dst, p_t)
                else:
                    eng.copy(dst, p_t)
                if hr % 4 == 3:  # row pair complete -> contiguous store
                    rp = row // 2
                    (nc.sync, nc.scalar)[rp % 2].dma_start(
                        td_d.ap()[:, 2 * rp : 2 * rp + 2, :, :],
                        s_T[:, 2 * rp : 2 * rp + 2, :, :],
                    )

            # --- five per-dy gathers into s_R[(h,t), tap, w] ---
            # td flat offset for (h,t,dy,dx,w):
            #   (5*dy+dx)*6912 + (h+dy)*576 + t*36 + (w+dx)
            s_R = sb.tile([128, NTAP, W], bf16)
            for dy in range(KS):
                src = bass.AP(
                    tensor=td_d.ap().tensor,
                    offset=dy * (5 * SLAB_R * S * SLAB_W + S * SLAB_W),
                    ap=[
                        [SLAB_W, 128],                       # (h,t) partitions
                        [SLAB_R * S * SLAB_W + 1, KS],       # dx
                        [1, W],                              # w
                    ],
                )
                (nc.sync, nc.scalar)[dy % 2].dma_start(
                    s_R[:, KS * dy : KS * (dy + 1), :], src
                )

            # --- tap reduce; exp; den/rcp/broadcasts via PE indicators ---
            s_sk = sb.tile([128, W], f32)
            nc.vector.tensor_reduce(
                s_sk, s_R.transpose([0, 2, 1]), axis=mybir.AxisListType.X, op=ALU.add
            )
            s_e = sb.tile([128, W], f32)
            nc.scalar.activation(s_e, s_sk, AF.Exp)
            p_den = pso.tile([ROWS, W], f32, tag="den")
            nc.tensor.matmul(p_den, s_hsel, s_e, start=True, stop=True)
            s_rcp = sb.tile([ROWS, W], f32)
            nc.vector.reciprocal(s_rcp, p_den)
            p_rcpb = pso.tile([128, W], f32, tag="rcpb")
            nc.tensor.matmul(p_rcpb, s_irx, s_rcp, start=True, stop=True)
            s_en = sb.tile([128, W], bf16)
            nc.vector.tensor_tensor(s_en, s_e, p_rcpb, op=ALU.mult)

            # --- bounce e/den to [t, (h,w)]; replicate over ci8 via PE ---
            dst = bass.AP(
                tensor=ed_d.ap().tensor,
                offset=0,
                ap=[[W, ROWS], [ROWS * W, S], [1, W]],
            )
            nc.sync.dma_start(dst, s_en)
            s_e2 = sb.tile([S, HW], bf16)
            nc.sync.dma_start(s_e2, ed_d.ap())
            p_eb = pso.tile([128, HW], f32, tag="eb")
            nc.tensor.matmul(p_eb, s_irep, s_e2, start=True, stop=True)
            s_eb = sb.tile([128, HW], bf16)
            nc.vector.tensor_copy(s_eb, p_eb)

            # --- V path: xa = xt * e_n; contract (ci8,t) on PE ---
            s_xa = sb.tile([128, 8, HW], bf16)
            nc.vector.tensor_tensor(
                s_xa,
                s_xt,
                s_eb.unsqueeze(1).broadcast_to((128, 8, HW)),
                op=ALU.mult,
            )
            p_o = pso.tile([C, HW], f32, tag="out")
            for g in range(8):
                nc.tensor.matmul(
                    p_o,
                    s_w1vr[:, g, :],
                    s_xa[:, g, :],
                    start=(g == 0),
                    stop=(g == 7),
                )
            s_o = sb.tile([C, HW], f32)
            nc.scalar.copy(s_o, p_o)
            nc.sync.dma_start(o_d.ap(), s_o)

    nc.compile()
    return nc


def _get_module():
    global _MODULE
    if _MODULE is None:
        _MODULE = _build_module()
    return _MODULE


def make_host_inputs(x, w1, b1, w2, b2):
    """Host-side precompute: folded weights + per-core reflect-padded slices."""
    import ml_dtypes

    bf16 = ml_dtypes.bfloat16
    x = np.ascontiguousarray(np.asarray(x, np.float32))
    w1 = np.asarray(w1, np.float32)
    w2 = np.asarray(w2, np.float32)

    w1K = w1[C : 2 * C, :, 0, 0]          # [c, ci]
    w2K = w2[0, C : 2 * C]                # [c, 5, 5]
    weff = np.ascontiguousarray(
        np.einsum("ci,cyx->iyx", w1K, w2K).reshape(C, NTAP)
    ).astype(bf16)
    w1V = w1[2 * C :, :, 0, 0]            # [co, ci]

    # w1vr[(ci8,t), g, co] = w1V[co, 8g+ci8]
    tmp = w1V.T.reshape(8, 8, C)                      # (g, ci8, co)
    w1vr = np.ascontiguousarray(
        np.broadcast_to(tmp[:, :, None, :], (8, 8, S, C))
        .transpose(1, 2, 0, 3)
        .reshape(128, 8, C)
    ).astype(bf16)

    # spatial partition packing is (h,t): p = h*16 + t
    hsel = np.zeros((128, ROWS), np.float32)
    for h in range(ROWS):
        hsel[h * S : (h + 1) * S, h] = 1.0
    irx = np.ascontiguousarray(hsel.T)                # [h', (h,t)]
    irep = np.zeros((S, 128), np.float32)             # [t', (ci8,t)]
    for t in range(S):
        irep[t, t::S] = 1.0
    irep = irep.astype(bf16)

    in_maps = []
    for core in range(NCORES):
        b, hc = divmod(core, 4)
        h0 = ROWS * hc
        xp = np.pad(x[b], ((0, 0), (PAD, PAD), (PAD, PAD), (0, 0)), mode="reflect")
        slab = np.ascontiguousarray(xp[:, h0 : h0 + SLAB_R, :, :]).astype(bf16)
        xs = x[b][:, h0 : h0 + ROWS, :, :]            # [ci, h, w, t]
        xt = np.ascontiguousarray(
            xs.reshape(8, 8, ROWS, W, S)
            .transpose(1, 4, 0, 2, 3)
            .reshape(128, 8, HW)
        ).astype(bf16)
        in_maps.append(
            {
                "slab": slab,
                "xt": xt,
                "weff": weff,
                "w1vr": w1vr,
                "hsel": hsel,
                "irx": irx,
                "irep": irep,
            }
        )
    return in_maps


def assemble_output(results, b1):
    b1V = np.asarray(b1, np.float32)[2 * C :]
    out = np.empty((B, C, H, W, S), np.float32)
    for core in range(NCORES):
        b, hc = divmod(core, 4)
        h0 = ROWS * hc
        o = results[core]["o"].reshape(C, ROWS, W).astype(np.float32)
        out[b, :, h0 : h0 + ROWS, :, :] = (
            o[:, :, :, None] + b1V[:, None, None, None]
        )
    return out


def kernel(x, w1, b1, w2, b2):
    from concourse.bass_utils import run_bass_kernel_spmd

    nc = _get_module()
    in_maps = make_host_inputs(x, w1, b1, w2, b2)
    res = run_bass_kernel_spmd(nc, in_maps, core_ids=list(range(NCORES)))
    return assemble_output(res.results, b1)


# revision 4
# speedup vs baseline: 1.8342x; 1.1944x over previous
"""Trainium2 Bass kernel for nn_ConvAttention_34600256537137.

Math notes (validated against the reference):
  qkv = 1x1conv(x, w1)+b1 -> Q,K,V;  score = conv5x5(Q_s)+conv5x5(K_t)+b2;
  attn = softmax_t(score);  out = einsum(attn, V).
  Softmax over t is shift-invariant, so the Q-half of the score (constant in
  t), b2, and the K-path bias all cancel.  The computation collapses to:
    weff[ci,dy,dx] = sum_c w1K[c,ci] * w2K[c,dy,dx]        (host, tiny)
    sK[b,t,h,w]    = conv5x5_reflect(x[b,:,:,:,t], weff)
    e = exp(sK);  den = sum_t e
    out[b,o,h,w,s] = (sum_{ci,t} w1V[o,ci] * e * x) / den + b1V[o]
  (s-independent; the S-broadcast and +b1V happen on host)

Sharding: 8 cores = (b in {0,1}) x (4 chunks of 8 rows of H).

Perf structure (v5):
  - all matmul operands are bf16: 1 cycle/row on PE (fp32 is 4) and half
    the HBM bytes for the two big loads (slab, xt).  rhs streams are kept
    contiguous (strided PE streams run ~4x slower).
  - conv: T[tap,(w,t)] = weff^T @ slab per half-row on PE; PSUM->SBUF
    copies (vector+scalar alternating) write s_T[tap, r, t, w] bf16;
    contiguous row-pair stores to td; five per-dy 3-dim-AP gathers build
    s_R[(h,t), tap, w]; per-dy partial reduces pipeline behind gathers.
  - spatial partition packing is (h,t): h=p//16, t=p%16.
  - e is bounced (8KB) to [t,(h,w)] and replicated over ci8 by an
    indicator matmul; den->rcp->rcpb (stride-0 broadcast DRAM read) runs
    in parallel and the normalization fuses into the output PSUM read.
  - gpsimd (software DGE) only carries small early constant loads; all
    critical DMAs ride the two HW DGE queues (sync/scalar).
"""

import sys

if "/opt/trn_rl_repo" not in sys.path:
    sys.path.insert(0, "/opt/trn_rl_repo")

import numpy as np

B, C, H, W, S = 2, 64, 32, 32, 16
KS, PAD = 5, 2
NCORES = 8
ROWS = H // 4            # output rows per core (8)
SLAB_R = ROWS + 2 * PAD  # 12
SLAB_W = W + 2 * PAD     # 36
NTAP = KS * KS           # 25
HW = ROWS * W            # 256 output positions
HREST = SLAB_W // 2      # 18
NPOS = SLAB_R * S * SLAB_W  # 6912

_MODULE = None


def _build_module():
    import concourse.bacc as bacc
    import concourse.bass as bass
    import concourse.tile as tile
    from concourse import mybir

    f32 = mybir.dt.float32
    bf16 = mybir.dt.bfloat16
    AF = mybir.ActivationFunctionType
    ALU = mybir.AluOpType
    nc = bacc.Bacc("TRN2", target_bir_lowering=False, debug=False, num_devices=NCORES)

    slab_d = nc.dram_tensor("slab", [C, SLAB_R, SLAB_W, S], bf16, kind="ExternalInput")
    xt_d = nc.dram_tensor("xt", [128, 8, HW], bf16, kind="ExternalInput")
    weff_d = nc.dram_tensor("weff", [C, NTAP], bf16, kind="ExternalInput")
    w1vr_d = nc.dram_tensor("w1vr", [128, 8, C], bf16, kind="ExternalInput")
    hsel_d = nc.dram_tensor("hsel", [128, ROWS], bf16, kind="ExternalInput")
    irep_d = nc.dram_tensor("irep", [S, 128], bf16, kind="ExternalInput")
    o_d = nc.dram_tensor("o", [C, HW], f32, kind="ExternalOutput")

    # scratch DRAM for partition-crossing rearrangements
    td_d = nc.dram_tensor("td", [NTAP, SLAB_R, S, SLAB_W], bf16)  # T, tap-major
    ed_d = nc.dram_tensor("ed", [S, ROWS, W], bf16)               # e, t-major
    rd_d = nc.dram_tensor("rd", [HW], f32)                        # 1/den, flat

    with tile.TileContext(nc) as tc:
        with tc.tile_pool(name="sb", bufs=1) as sb, tc.tile_pool(
            name="ps", bufs=4, space="PSUM"
        ) as ps, tc.tile_pool(name="pso", bufs=1, space="PSUM") as pso:
            # --- loads ---
            s_weff = sb.tile([C, NTAP], bf16)
            nc.sync.dma_start(s_weff, weff_d.ap())
            slab_t = []
            for rp in range(6):
                st = sb.tile([C, 2, SLAB_W, S], bf16, tag=f"slab{rp}")
                slab_t.append(st)
                (nc.sync, nc.scalar)[rp % 2].dma_start(
                    st, slab_d.ap()[:, 2 * rp : 2 * rp + 2, :, :]
                )
            s_hsel = sb.tile([128, ROWS], bf16)
            nc.gpsimd.dma_start(s_hsel, hsel_d.ap())
            s_irep = sb.tile([S, 128], bf16)
            nc.gpsimd.dma_start(s_irep, irep_d.ap())
            s_w1vr = sb.tile([128, 8, C], bf16)
            nc.gpsimd.dma_start(s_w1vr, w1vr_d.ap())
            s_xt = sb.tile([128, 8, HW], bf16)
            nc.scalar.dma_start(s_xt, xt_d.ap())

            # --- phase 1: T = weff^T @ slab, half-row chunks, contiguous
            # rhs stream (w,t); the PSUM->SBUF copy applies the (w,t)->(t,w)
            # transpose ---
            s_T = sb.tile([NTAP, SLAB_R, S, SLAB_W], bf16)
            for hr in range(SLAB_R * 2):
                row, half = divmod(hr, 2)
                p_t = ps.tile([NTAP, HREST, S], f32, tag="pt")
                nc.tensor.matmul(
                    p_t,
                    s_weff,
                    slab_t[row // 2][:, row % 2, half * HREST : (half + 1) * HREST, :],
                    start=True,
                    stop=True,
                )
                eng = nc.vector if hr % 2 == 0 else nc.scalar
                dst = s_T[:, row, :, half * HREST : (half + 1) * HREST]
                if eng is nc.vector:
                    eng.tensor_copy(dst, p_t.transpose([0, 2, 1]))
                else:
                    eng.copy(dst, p_t.transpose([0, 2, 1]))
                if hr % 4 == 3:  # row pair complete -> contiguous store
                    rp = row // 2
                    nc.sync.dma_start(
                        td_d.ap()[:, 2 * rp : 2 * rp + 2, :, :],
                        s_T[:, 2 * rp : 2 * rp + 2, :, :],
                    )

            # --- five per-dy gathers into s_R[(h,t), tap, w], with per-dy
            # partial tap reduces pipelined behind them ---
            # td flat offset for (h,t,dy,dx,w):
            #   (5*dy+dx)*6912 + (h+dy)*576 + t*36 + (w+dx)
            s_R = sb.tile([128, NTAP, W], bf16)
            for dy in range(KS):
                src = bass.AP(
                    tensor=td_d.ap().tensor,
                    offset=dy * (5 * NPOS + S * SLAB_W),
                    ap=[
                        [SLAB_W, 128],       # (h,t) partitions
                        [NPOS + 1, KS],      # dx
                        [1, W],              # w
                    ],
                )
                (nc.sync, nc.scalar)[dy % 2].dma_start(
                    s_R[:, KS * dy : KS * (dy + 1), :], src
                )
            s_pd = []
            for dy in range(KS):
                p = sb.tile([128, W], f32, tag=f"pd{dy}")
                nc.vector.tensor_reduce(
                    p,
                    s_R[:, KS * dy : KS * (dy + 1), :].transpose([0, 2, 1]),
                    axis=mybir.AxisListType.X,
                    op=ALU.add,
                )
                s_pd.append(p)
                if dy == 1:
                    s_a01 = sb.tile([128, W], f32)
                    nc.vector.tensor_tensor(s_a01, s_pd[0], s_pd[1], op=ALU.add)
                if dy == 3:
                    s_a03 = sb.tile([128, W], f32)
                    nc.vector.tensor_tensor(s_a03, s_pd[2], s_pd[3], op=ALU.add)
                    nc.vector.tensor_tensor(s_a03, s_a01, s_a03, op=ALU.add)
            s_sk = sb.tile([128, W], f32)
            nc.vector.tensor_tensor(s_sk, s_a03, s_pd[4], op=ALU.add)

            # --- e = exp(sK); den path runs in parallel with the e bounce ---
            s_e = sb.tile([128, W], bf16)
            nc.scalar.activation(s_e, s_sk, AF.Exp)
            # e bounce to [t, (h,w)] (critical path, sync queue)
            dst = bass.AP(
                tensor=ed_d.ap().tensor,
                offset=0,
                ap=[[W, ROWS], [ROWS * W, S], [1, W]],
            )
            nc.sync.dma_start(dst, s_e)
            s_e2 = sb.tile([S, HW], bf16)
            nc.sync.dma_start(s_e2, ed_d.ap())
            # den path (parallel, scalar queue)
            p_den = pso.tile([ROWS, W], f32, tag="den")
            nc.tensor.matmul(p_den, s_hsel, s_e, start=True, stop=True)
            s_rcp = sb.tile([ROWS, W], f32)
            nc.vector.reciprocal(s_rcp, p_den)
            nc.scalar.dma_start(rd_d.ap(), s_rcp)
            s_rcpb = sb.tile([C, HW], f32)
            nc.scalar.dma_start(
                s_rcpb,
                bass.AP(tensor=rd_d.ap().tensor, offset=0, ap=[[0, C], [1, HW]]),
            )

            # --- replicate e over ci8 via indicator matmul ---
            p_eb = pso.tile([128, HW], f32, tag="eb")
            nc.tensor.matmul(p_eb, s_irep, s_e2, start=True, stop=True)
            s_eb = sb.tile([128, HW], bf16)
            nc.vector.tensor_copy(s_eb, p_eb)

            # --- V path: xa = xt * e (two halves overlap the V matmuls);
            # contract (ci8,t) on PE; normalize on the PSUM read ---
            s_xa = sb.tile([128, 8, HW], bf16)
            p_o = pso.tile([C, HW], f32, tag="out")
            for halfg in range(2):
                nc.vector.tensor_tensor(
                    s_xa[:, 4 * halfg : 4 * halfg + 4, :],
                    s_xt[:, 4 * halfg : 4 * halfg + 4, :],
                    s_eb.unsqueeze(1).broadcast_to((128, 4, HW)),
                    op=ALU.mult,
                )
                for g in range(4 * halfg, 4 * halfg + 4):
                    nc.tensor.matmul(
                        p_o,
                        s_w1vr[:, g, :],
                        s_xa[:, g, :],
                        start=(g == 0),
                        stop=(g == 7),
                    )
            s_o = sb.tile([C, HW], f32)
            nc.vector.tensor_tensor(s_o, p_o, s_rcpb, op=ALU.mult)
            nc.sync.dma_start(o_d.ap(), s_o)

    nc.compile()
    return nc


def _get_module():
    global _MODULE
    if _MODULE is None:
        _MODULE = _build_module()
    return _MODULE


def make_host_inputs(x, w1, b1, w2, b2):
    """Host-side precompute: folded weights + per-core reflect-padded slices."""
    import ml_dtypes

    bf16 = ml_dtypes.bfloat16
    x = np.ascontiguousarray(np.asarray(x, np.float32))
    w1 = np.asarray(w1, np.float32)
    w2 = np.asarray(w2, np.float32)

    w1K = w1[C : 2 * C, :, 0, 0]          # [c, ci]
    w2K = w2[0, C : 2 * C]                # [c, 5, 5]
    weff = np.ascontiguousarray(
        np.einsum("ci,cyx->iyx", w1K, w2K).reshape(C, NTAP)
    ).astype(bf16)
    w1V = w1[2 * C :, :, 0, 0]            # [co, ci]

    # w1vr[(ci8,t), g, co] = w1V[co, 8g+ci8]
    tmp = w1V.T.reshape(8, 8, C)                      # (g, ci8, co)
    w1vr = np.ascontiguousarray(
        np.broadcast_to(tmp[:, :, None, :], (8, 8, S, C))
        .transpose(1, 2, 0, 3)
        .reshape(128, 8, C)
    ).astype(bf16)

    # spatial partition packing is (h,t): p = h*16 + t
    hsel = np.zeros((128, ROWS), np.float32)
    for h in range(ROWS):
        hsel[h * S : (h + 1) * S, h] = 1.0
    hsel = hsel.astype(bf16)
    irep = np.zeros((S, 128), np.float32)             # [t', (ci8,t)]
    for t in range(S):
        irep[t, t::S] = 1.0
    irep = irep.astype(bf16)

    in_maps = []
    for core in range(NCORES):
        b, hc = divmod(core, 4)
        h0 = ROWS * hc
        xp = np.pad(x[b], ((0, 0), (PAD, PAD), (PAD, PAD), (0, 0)), mode="reflect")
        slab = np.ascontiguousarray(xp[:, h0 : h0 + SLAB_R, :, :]).astype(bf16)
        xs = x[b][:, h0 : h0 + ROWS, :, :]            # [ci, h, w, t]
        xt = np.ascontiguousarray(
            xs.reshape(8, 8, ROWS, W, S)
            .transpose(1, 4, 0, 2, 3)
            .reshape(128, 8, HW)
        ).astype(bf16)
        in_maps.append(
            {
                "slab": slab,
                "xt": xt,
                "weff": weff,
                "w1vr": w1vr,
                "hsel": hsel,
                "irep": irep,
            }
        )
    return in_maps


def assemble_output(results, b1):
    b1V = np.asarray(b1, np.float32)[2 * C :]
    out = np.empty((B, C, H, W, S), np.float32)
    for core in range(NCORES):
        b, hc = divmod(core, 4)
        h0 = ROWS * hc
        o = results[core]["o"].reshape(C, ROWS, W).astype(np.float32)
        out[b, :, h0 : h0 + ROWS, :, :] = (
            o[:, :, :, None] + b1V[:, None, None, None]
        )
    return out


def kernel(x, w1, b1, w2, b2):
    from concourse.bass_utils import run_bass_kernel_spmd

    nc = _get_module()
    in_maps = make_host_inputs(x, w1, b1, w2, b2)
    res = run_bass_kernel_spmd(nc, in_maps, core_ids=list(range(NCORES)))
    return assemble_output(res.results, b1)
